# revision 13
# baseline (speedup 1.0000x reference)
"""Trainium2 Bass kernel for nn_DecoderBlock (B=4, S=2048, E=1024, H=16, D=64).

Sharding: 8 cores = 4 batches x 2 sequence-halves. Each core owns 1024 query
positions of one batch (a balanced causal split: core-even takes q [0:512)+
[1536:2048), core-odd takes q [512:1536)) and recomputes full-S K/V for its
batch locally (no collectives). Proj + FFN are token-parallel on the owned
1024 positions. Everything on-chip is in transposed layout (feature dim on
partitions); the host pre-transposes x and re-transposes the output.

The per-core program is identical (SPMD); per-core differences (which q
columns, causal masks) are encoded in the host-prepared inputs: xT columns
are reordered to [own-q | other-q], and causal masks are shipped per-core.
"""

import numpy as np
import ml_dtypes

import concourse.bass as bass
import concourse.tile as tile
from concourse import bacc, mybir
from concourse.bass_utils import run_bass_kernel_spmd

B, S, E, H, D = 4, 2048, 1024, 16, 64
QC = 1024          # queries owned per core
CH = 512           # q-chunk (matmul moving dim)
ET = E // 128      # 8 e-tiles
HT = (4 * E) // 128  # 32 ffn hidden tiles
SCALE = float(E) ** -0.5

F32R = mybir.dt.float32r
F32 = mybir.dt.float32
BF16 = mybir.dt.bfloat16

# Attention slot tables: (t_tile, mask_idx or None); uniform across cores.
# xT t-order is [own qA | own qB | other qA | other qB] (512 cols each).
# Each slot computes BOTH heads of a pair concurrently via PE row-groups
# (rows 0-63 / 64-127) into one [128,1024] psum tile -> one exp op.
CHUNK_A = [(0, 0), (1, 1), (2, 2), (3, 3), (8, 4), (9, 5), (10, 6), (11, 7)]
CHUNK_B = [(0, None), (1, None), (2, None), (3, None),
           (4, 8), (5, 9), (6, 10), (7, 11),
           (8, None), (9, None), (10, None), (11, None),
           (12, 12), (13, 13), (14, 14), (15, 15)]
N_MASKS = 16

_CACHE = {}
LAST_RESULTS = None


def _phase1_attention(nc, tc, xt, at, mk, dram, wkq_pool, wv_pool, pre):
    with (
        tc.tile_pool(name="kt", bufs=2) as kt_pool,
        tc.tile_pool(name="qt", bufs=2) as qt_pool,
        tc.tile_pool(name="vt", bufs=1) as vt_pool,
        tc.tile_pool(name="es", bufs=3) as es_pool,
        tc.tile_pool(name="norm", bufs=2) as nm_pool,
        tc.tile_pool(name="ps_kqv", bufs=2, space="PSUM") as pp_kqv,
        tc.tile_pool(name="ps_s", bufs=2, space="PSUM") as pp_s,
        tc.tile_pool(name="ps_av", bufs=2, space="PSUM") as pp_av,
    ):
        wk_d, wq_d, wv_d = dram["wk"], dram["wq"], dram["wv"]

        for g in range(4):  # head groups of 4
            kts, qts = [], []
            for pl in range(2):
                p = 2 * g + pl
                if ("wk", p) in pre:
                    wkt = pre["wk", p]
                else:
                    wkt = wkq_pool.tile([128, ET, 128], BF16, tag="w")
                    nc.sync.dma_start(wkt[:], wk_d[p])
                kt = kt_pool.tile([128, S], BF16)
                for tcnk in range(4):
                    ps = pp_kqv.tile([128, CH], F32)
                    for et in range(ET):
                        nc.tensor.matmul(
                            ps[:], wkt[:, et, :],
                            xt[:, et, tcnk * CH:(tcnk + 1) * CH],
                            start=(et == 0), stop=(et == ET - 1))
                    nc.vector.tensor_copy(kt[:, tcnk * CH:(tcnk + 1) * CH], ps[:])
                kts.append(kt)

                if ("wq", p) in pre:
                    wqt = pre["wq", p]
                else:
                    wqt = wkq_pool.tile([128, ET, 128], BF16, tag="w")
                    nc.sync.dma_start(wqt[:], wq_d[p])
                qt = qt_pool.tile([128, QC], BF16)
                for c in range(2):
                    ps = pp_kqv.tile([128, CH], F32)
                    for et in range(ET):
                        nc.tensor.matmul(
                            ps[:], wqt[:, et, :],
                            xt[:, et, c * CH:(c + 1) * CH],
                            start=(et == 0), stop=(et == ET - 1))
                    nc.vector.tensor_copy(qt[:, c * CH:(c + 1) * CH], ps[:])
                qts.append(qt)

            if ("wv", g) in pre:
                wvt = pre["wv", g]
            else:
                wvt = wv_pool.tile([128, ET, 256], BF16)
                nc.sync.dma_start(wvt[:], wv_d[g])
            vt = vt_pool.tile([128, 16, 4, 65], BF16)
            nc.vector.memset(vt[:, :, :, 64:65], 1.0)
            for tt in range(16):
                ps = pp_kqv.tile([128, CH], F32)
                for et in range(ET):
                    nc.tensor.matmul(
                        ps[:, 0:256], xt[:, et, tt * 128:(tt + 1) * 128],
                        wvt[:, et, :],
                        start=(et == 0), stop=(et == ET - 1))
                nc.vector.tensor_copy(
                    vt[:, tt, :, 0:64],
                    ps[:, 0:256].rearrange("p (g d) -> p g d", g=4))

            for hp in range(2):  # head pairs; two heads run concurrently
                kt, qt = kts[hp], qts[hp]
                hl0, hl1 = 2 * hp, 2 * hp + 1
                tile_j = 2 * g + hp  # attnT e-tile index for this pair
                for c, slots in ((0, CHUNK_A), (1, CHUNK_B)):
                    av0 = pp_av.tile([65, CH], F32, tag="av")
                    av1 = pp_av.tile([65, CH], F32, tag="av")
                    n = len(slots)
                    for si, (tt, mi) in enumerate(slots):
                        ps = pp_s.tile([128, 2 * CH], F32)
                        for half, r0 in ((0, 0), (1, 64)):
                            nc.tensor.matmul(
                                ps[:, half * CH:(half + 1) * CH],
                                kt[r0:r0 + 64, tt * 128:(tt + 1) * 128],
                                qt[r0:r0 + 64, c * CH:(c + 1) * CH],
                                start=True, stop=True)
                        es = es_pool.tile([128, 2 * CH], BF16)
                        nc.scalar.activation(
                            es[:], ps[:], mybir.ActivationFunctionType.Exp,
                            scale=SCALE)
                        if mi is not None:
                            nc.vector.tensor_mul(
                                es[:, 0:CH], es[:, 0:CH], mk[:, mi, :])
                            nc.vector.tensor_mul(
                                es[:, CH:2 * CH], es[:, CH:2 * CH], mk[:, mi, :])
                        nc.tensor.matmul(
                            av0[:], vt[:, tt, hl0, :], es[:, 0:CH],
                            start=(si == 0), stop=(si == n - 1))
                        nc.tensor.matmul(
                            av1[:], vt[:, tt, hl1, :], es[:, CH:2 * CH],
                            start=(si == 0), stop=(si == n - 1))
                    for half, avx in ((0, av0), (1, av1)):
                        rc = nm_pool.tile([1, CH], F32, tag="rc")
                        nc.vector.reciprocal(rc[:], avx[64:65, :])
                        bs = nm_pool.tile([64, CH], F32, tag="bs")
                        nc.gpsimd.partition_broadcast(bs[:], rc[:])
                        nc.vector.tensor_mul(
                            at[64 * half:64 * half + 64, tile_j,
                               c * CH:(c + 1) * CH],
                            avx[0:64, :], bs[:])


def _phase2_proj(nc, tc, xr, x1b, at, wo, bo):
    """x1 = x + attn @ Wo + bo, written in place into xr."""
    with tc.tile_pool(name="ps_y", bufs=4, space="PSUM") as pp_y:
        for j in range(ET):
            for c in range(2):
                ps = pp_y.tile([128, CH], F32)
                for i in range(ET):
                    nc.tensor.matmul(
                        ps[:], wo[:, i, j, :], at[:, i, c * CH:(c + 1) * CH],
                        start=(i == 0), stop=(i == ET - 1))
                nc.vector.scalar_tensor_tensor(
                    xr[:, j, c * CH:(c + 1) * CH], ps[:], bo[:, j, :],
                    xr[:, j, c * CH:(c + 1) * CH],
                    op0=mybir.AluOpType.add, op1=mybir.AluOpType.add)
                nc.vector.tensor_copy(
                    x1b[:, j, c * CH:(c + 1) * CH],
                    xr[:, j, c * CH:(c + 1) * CH])


def _phase3_ffn(nc, tc, xr, x1b, out_d, dram):
    w1_d, w2_d = dram["w1"], dram["w2"]
    with (
        tc.tile_pool(name="hT", bufs=2) as h_pool,
        tc.tile_pool(name="w1s", bufs=3) as w1_pool,
        tc.tile_pool(name="w2s", bufs=2) as w2_pool,
        tc.tile_pool(name="b12", bufs=1) as b12_pool,
        tc.tile_pool(name="osb", bufs=3) as o_pool,
        tc.tile_pool(name="ps_h", bufs=4, space="PSUM") as pp_h,
        tc.tile_pool(name="ps_f", bufs=4, space="PSUM") as pp_f,
    ):
        b1 = b12_pool.tile([128, HT, 1], F32, tag="b1")
        nc.sync.dma_start(b1[:], dram["b1"][:])
        b2 = b12_pool.tile([128, ET, 1], F32, tag="b2")
        nc.sync.dma_start(b2[:], dram["b2"][:])
        hT0 = h_pool.tile([128, HT, CH], BF16, tag="hT")
        hT1 = h_pool.tile([128, HT, CH], BF16, tag="hT")
        hts = [hT0, hT1]
        for t in range(HT):
            w1t = w1_pool.tile([128, ET, 128], BF16)
            nc.sync.dma_start(w1t[:], w1_d[t])
            for c in range(2):
                ps = pp_h.tile([128, CH], F32)
                for i in range(ET):
                    nc.tensor.matmul(
                        ps[:], w1t[:, i, :], x1b[:, i, c * CH:(c + 1) * CH],
                        start=(i == 0), stop=(i == ET - 1))
                nc.scalar.activation(
                    hts[c][:, t, :], ps[:], mybir.ActivationFunctionType.Relu,
                    bias=b1[:, t, :])
        for j in range(ET):
            w2t = w2_pool.tile([128, HT, 128], BF16)
            nc.sync.dma_start(w2t[:], w2_d[j])
            for c in range(2):
                ps = pp_f.tile([128, CH], F32)
                for t in range(HT):
                    nc.tensor.matmul(
                        ps[:], w2t[:, t, :], hts[c][:, t, :],
                        start=(t == 0), stop=(t == HT - 1))
                ot = o_pool.tile([128, CH], BF16)
                nc.vector.scalar_tensor_tensor(
                    ot[:], ps[:], b2[:, j, :], xr[:, j, c * CH:(c + 1) * CH],
                    op0=mybir.AluOpType.add, op1=mybir.AluOpType.add)
                nc.sync.dma_start(out_d[j][:, c * CH:(c + 1) * CH], ot[:])


def build_nc(reps=1, phases=(1, 2, 3)):
    nc = bacc.Bacc("TRN2", target_bir_lowering=False, debug=False, num_devices=8)

    dram = {}
    dram["xT"] = nc.declare_dram_parameter("xT", [128, ET, S], BF16, isOutput=False)
    dram["xr"] = nc.declare_dram_parameter("xr", [128, ET, QC], F32, isOutput=False)
    dram["wq"] = nc.declare_dram_parameter("wq", [8, 128, ET, 128], BF16, isOutput=False)
    dram["wk"] = nc.declare_dram_parameter("wk", [8, 128, ET, 128], BF16, isOutput=False)
    dram["wv"] = nc.declare_dram_parameter("wv", [4, 128, ET, 256], BF16, isOutput=False)
    dram["wo"] = nc.declare_dram_parameter("wo", [128, ET, ET, 128], BF16, isOutput=False)
    dram["w1"] = nc.declare_dram_parameter("w1", [HT, 128, ET, 128], BF16, isOutput=False)
    dram["w2"] = nc.declare_dram_parameter("w2", [ET, 128, HT, 128], BF16, isOutput=False)
    dram["bo"] = nc.declare_dram_parameter("bo", [128, ET, 1], F32, isOutput=False)
    dram["b1"] = nc.declare_dram_parameter("b1", [128, HT, 1], F32, isOutput=False)
    dram["b2"] = nc.declare_dram_parameter("b2", [128, ET, 1], F32, isOutput=False)
    dram["masks"] = nc.declare_dram_parameter(
        "masks", [128, N_MASKS, CH], BF16, isOutput=False)
    out_d = nc.declare_dram_parameter("outT", [ET, 128, QC], BF16, isOutput=True)

    with tile.TileContext(nc) as tc:
        for _rep in range(reps):
            with (
                tc.tile_pool(name="xt", bufs=1) as xt_pool,
                tc.tile_pool(name="xr", bufs=1) as xr_pool,
                tc.tile_pool(name="x1b", bufs=1) as x1b_pool,
                tc.tile_pool(name="masks", bufs=1) as mk_pool,
            ):
                xt = xt_pool.tile([128, ET, S], BF16)
                xr = xr_pool.tile([128, ET, QC], F32)
                x1b = x1b_pool.tile([128, ET, QC], BF16)
                mk = mk_pool.tile([128, N_MASKS, CH], BF16)
                with (
                    tc.tile_pool(name="wkq", bufs=3) as wkq_pool,
                    tc.tile_pool(name="wv", bufs=1) as wv_pool,
                    tc.tile_pool(name="attnT", bufs=1) as at_pool,
                    tc.tile_pool(name="wo", bufs=1) as wo_pool,
                    tc.tile_pool(name="bo", bufs=1) as bo_pool,
                ):
                    # DMA priority order: first K matmuls need only xt chunk
                    # 0 + wk plane 0, so emit those first; masks and
                    # phase-2-only inputs last.
                    pre = {}
                    wk0 = wkq_pool.tile([128, ET, 128], BF16, tag="w")
                    pre["wk", 0] = wk0
                    nc.sync.dma_start(wk0[:], dram["wk"][0])
                    for et in range(ET):
                        nc.sync.dma_start(
                            xt[:, et, 0:CH], dram["xT"][:, et, 0:CH])
                    nc.sync.dma_start(
                        xt[:, :, CH:2 * CH], dram["xT"][:, :, CH:2 * CH])
                    wq0 = wkq_pool.tile([128, ET, 128], BF16, tag="w")
                    pre["wq", 0] = wq0
                    nc.sync.dma_start(wq0[:], dram["wq"][0])
                    for sc in range(2, 4):
                        nc.sync.dma_start(
                            xt[:, :, sc * CH:(sc + 1) * CH],
                            dram["xT"][:, :, sc * CH:(sc + 1) * CH])
                    nc.sync.dma_start(
                        mk[:, 0:8, :], dram["masks"][:, 0:8, :])
                    nc.sync.dma_start(
                        mk[:, 8:16, :], dram["masks"][:, 8:16, :])
                    at = at_pool.tile([128, ET, QC], BF16)
                    wo = wo_pool.tile([128, ET, ET, 128], BF16)
                    bo = bo_pool.tile([128, ET, 1], F32)
                    if 1 in phases:
                        _phase1_attention(nc, tc, xt, at, mk, dram,
                                          wkq_pool, wv_pool, pre)
                    nc.sync.dma_start(xr[:], dram["xr"][:])
                    nc.sync.dma_start(wo[:], dram["wo"][:])
                    nc.sync.dma_start(bo[:], dram["bo"][:])
                    if 2 in phases:
                        _phase2_proj(nc, tc, xr, x1b, at, wo, bo)
                if 3 in phases:
                    if 2 not in phases:
                        nc.sync.dma_start(
                            x1b[:], dram["xT"][:, :, 0:QC])
                    _phase3_ffn(nc, tc, xr, x1b, out_d, dram)

    nc.compile()
    return nc


def _qsel(half):
    if half == 0:
        return np.concatenate([np.arange(0, 512), np.arange(1536, 2048)])
    return np.arange(512, 1536)


def make_masks(half):
    """bf16 [128, 16, 1024] per-core causal keep-masks (dup for head pair)."""
    own = _qsel(half)
    other = _qsel(1 - half)
    tpos = np.concatenate([own, other])          # actual seq position per t col
    qpos = own
    m = np.zeros((N_MASKS, 128, CH), dtype=np.float32)
    for slots, q0 in ((CHUNK_A, 0), (CHUNK_B, 512)):
        for tt, mi in slots:
            if mi is None:
                continue
            q_act = qpos[q0:q0 + CH]
            t_act = tpos[tt * 128:(tt + 1) * 128]
            m[mi] = (t_act[:, None] <= q_act[None, :]).astype(np.float32)
    return np.ascontiguousarray(m.transpose(1, 0, 2)).astype(ml_dtypes.bfloat16)


def prep_shared(Wq, Wk, Wv, Wo, bo, W1, b1, W2, b2):
    f = np.float32
    wq = np.stack([Wq[2 * p:2 * p + 2].transpose(1, 0, 2).reshape(E, 128)
                   .reshape(ET, 128, 128).transpose(1, 0, 2) for p in range(8)])
    wk = np.stack([Wk[2 * p:2 * p + 2].transpose(1, 0, 2).reshape(E, 128)
                   .reshape(ET, 128, 128).transpose(1, 0, 2) for p in range(8)])
    wv = np.stack([Wv[4 * g:4 * g + 4].transpose(1, 0, 2).reshape(E, 256)
                   .reshape(ET, 128, 256).transpose(1, 0, 2) for g in range(4)])
    wo = Wo.reshape(ET, 128, ET, 128).transpose(1, 0, 2, 3)
    w1 = W1.reshape(ET, 128, HT, 128).transpose(2, 1, 0, 3)
    w2 = W2.reshape(HT, 128, ET, 128).transpose(2, 1, 0, 3)
    return {
        "wq": np.ascontiguousarray(wq).astype(ml_dtypes.bfloat16),
        "wk": np.ascontiguousarray(wk).astype(ml_dtypes.bfloat16),
        "wv": np.ascontiguousarray(wv).astype(ml_dtypes.bfloat16),
        "wo": np.ascontiguousarray(wo).astype(ml_dtypes.bfloat16),
        "w1": np.ascontiguousarray(w1).astype(ml_dtypes.bfloat16),
        "w2": np.ascontiguousarray(w2).astype(ml_dtypes.bfloat16),
        "bo": np.ascontiguousarray(bo.reshape(ET, 128, 1).transpose(1, 0, 2)).astype(f),
        "b1": np.ascontiguousarray(b1.reshape(HT, 128, 1).transpose(1, 0, 2)).astype(f),
        "b2": np.ascontiguousarray(b2.reshape(ET, 128, 1).transpose(1, 0, 2)).astype(f),
    }


def make_in_maps(x, Wq, Wk, Wv, Wo, bo, W1, b1, W2, b2):
    shared = prep_shared(Wq, Wk, Wv, Wo, bo, W1, b1, W2, b2)
    masks = [make_masks(half) for half in range(2)]
    in_maps = []
    for core in range(8):
        b, half = core // 2, core % 2
        own = _qsel(half)
        torder = np.concatenate([own, _qsel(1 - half)])
        xTc = np.ascontiguousarray(np.asarray(x[b]).T[:, torder]
                                   .reshape(ET, 128, S).transpose(1, 0, 2))
        in_maps.append({"xT": xTc.astype(ml_dtypes.bfloat16),
                        "xr": xTc[:, :, 0:QC].astype(np.float32),
                        "masks": masks[half], **shared})
    return in_maps


def kernel(**inputs):
    global LAST_RESULTS
    if "nc" not in _CACHE:
        _CACHE["nc"] = build_nc()
    nc = _CACHE["nc"]
    in_maps = make_in_maps(
        inputs["x"], inputs["Wq"], inputs["Wk"], inputs["Wv"], inputs["Wo"],
        inputs["bo"], inputs["W1"], inputs["b1"], inputs["W2"], inputs["b2"])
    res = run_bass_kernel_spmd(nc, in_maps, list(range(8)))
    LAST_RESULTS = res
    out = np.empty((B, S, E), dtype=np.float32)
    for core in range(8):
        b, half = core // 2, core % 2
        outT = res.results[core]["outT"].reshape(E, QC).astype(np.float32)
        out[b, _qsel(half), :] = outT.T
    return out

